# revision 25
# baseline (speedup 1.0000x reference)
"""Bass/Tile kernel for BilinearAttentionLayer on 8 NeuronCores.

out[b] = softmax(x[b] @ W @ x[b]^T / sqrt(D)) @ x[b]

Sharding: data-parallel over batch (8 batches -> 8 cores). Each core runs an
identical program on its own batch slice (x[b], W) -> out[b].

Per-core layout: the score matrix is kept transposed (scoresT[t, s]) so
every matmul operand is naturally oriented and no score-matrix transposes
are needed:
  xwT   = matmul(lhsT=W,   rhs=xT)      [e, s]
  prodT = matmul(lhsT=xT,  rhs=xwT)     [t, s]   (softmax axis = partitions)
  PT    = exp((prodT - rowmax)/sqrt(D))           (exact, safe softmax)
  out   = matmul(lhsT=PT,  rhs=x_nat)   [s, d]   (lands natural)
  rowsum rides the PV loop as N=1 matmuls against a ones column, landing
  directly in [s-partition, 1] layout for the per-partition normalization.
The only data transposes are 64 PE transposes of x itself.

Row max (softmax axis = partitions here): DVE max-accumulate across the 16
prodT PSUM tiles -> one GpSimd partition_all_reduce(max), whose output is
already replicated across all partitions -> DVE subtract on the staged raw
scores -> ScalarE Exp.  The per-row max makes the kernel robust to any
input values (the shifted exp never overflows and rowsum >= 1).

Dtypes: score path (xT, W, xwT) in float32r (1 cycle/row on the PE at
N=512); raw scores staged in f32; PT and x_nat in bf16 for the PV matmul.

Scheduling (what this revision changes vs. the first working version):
 * Input DMAs are issued up front and split across BOTH hardware DGE
   queues (SP + Activation) -- descriptor generation was the input-load
   bottleneck (2KB per descriptor, ~10 ns per descriptor per DGE).
   W goes first on the Act queue, x tiles alternate SP/Act.
 * Transposes and xw chunks are interleaved so the PE stream chases the
   DMA stream tile-by-tile and the HAM clock never re-throttles.
 * Each PV block (stage_b) is zipped into the NEXT prod block's matmul
   stream at per-tile granularity, so the ScalarE/DVE softmax work of one
   block spreads across the whole PE stream instead of bunching up and
   stalling the PV matmuls at block boundaries.
 * The last block's sub/exp is spread evenly across the previous block's
   PV stream (the old version bunched it into the first quarter and the
   final PV stalled ~5us on the exp backlog).
 * Output normalize runs on ScalarE (activation Copy with per-partition
   reciprocal scale) straight out of PSUM; output DMAs alternate queues.
"""

import numpy as np

import concourse.mybir as mybir
import concourse.tile as tile
from concourse import bacc
from concourse import bass_isa
from concourse import bass_utils
from concourse.masks import make_identity

B = 8
S = 2048
D = 512
P = 128
SB = 512  # s-block width (one fp32 PSUM bank)

F32 = mybir.dt.float32
F32R = mybir.dt.float32r
BF16 = mybir.dt.bfloat16

SCALE = float(1.0 / np.sqrt(np.float64(D)))
AF = mybir.ActivationFunctionType


def build_nc(s=S, d=D):
    nd = d // P   # d/e tiles of 128
    nst = s // P  # s/t tiles of 128
    nsb = s // SB  # s-blocks
    nss = SB // P  # 128-chunks per s-block

    nc = bacc.Bacc(
        "TRN2",
        target_bir_lowering=False,
        debug=False,
        num_devices=B,
    )
    x_d = nc.dram_tensor("x", [s, d], F32, kind="ExternalInput").ap()
    w_d = nc.dram_tensor("w", [d, d], F32, kind="ExternalInput").ap()
    o_d = nc.dram_tensor("o", [s, d], F32, kind="ExternalOutput").ap()

    x_tiled = x_d.rearrange("(n p) d -> p n d", p=P)  # [128, nst, d]
    w_tiled = w_d.rearrange("(k p) e -> p k e", p=P)  # [128, nd, d]
    o_tiled = o_d.rearrange("(n p) d -> p n d", p=P)

    with tile.TileContext(nc) as tc:
        with (
            tc.tile_pool(name="const", bufs=1) as constp,
            tc.tile_pool(name="big", bufs=1) as bigp,
            tc.tile_pool(name="strip", bufs=2) as stripp,
            tc.tile_pool(name="ptp", bufs=2) as ptp,
            tc.tile_pool(name="bcast", bufs=2) as bcp,
            tc.tile_pool(name="outs", bufs=3) as outp,
            tc.tile_pool(name="acc", bufs=2) as accp,
            tc.tile_pool(name="rsacc", bufs=2) as rsaccp,
            tc.tile_pool(name="rrsb", bufs=2) as rrsbp,
            tc.tile_pool(name="mm", bufs=6, space="PSUM") as mmp,
            tc.tile_pool(name="tr", bufs=2, space="PSUM") as trp,
        ):
            ident = constp.tile([P, P], F32)
            make_identity(nc, ident[:])
            x_nat = bigp.tile([P, nst, d], BF16)
            xT = bigp.tile([P, nd, s], F32R)
            w_sb = bigp.tile([P, nd, d], F32R)
            xwT = bigp.tile([P, nd, s], F32R)

            # ---- staging: x and W land in the strip pool's two buffers
            # (same shape, dead before the strips are first written), so
            # every input DMA can be issued up front with no buffer
            # interlock, split across both HWDGE queues.
            x_stage = stripp.tile([P, nst, SB], F32, tag="strip", name="xstg")
            w_stage = stripp.tile([P, nst, SB], F32, tag="strip", name="wstg")
            # x0/x1 first so the transposes can start ASAP; W right after
            # (needed by xw_chunk(0)); the rest of x streams behind.
            nc.sync.dma_start(x_stage[:, 0, :], x_tiled[:, 0, :])
            nc.scalar.dma_start(x_stage[:, 1, :], x_tiled[:, 1, :])
            nc.sync.dma_start(w_stage[:, 0:2, :], w_tiled[:, 0:2, :])
            nc.scalar.dma_start(w_stage[:, 2:4, :], w_tiled[:, 2:4, :])
            for st in range(2, nst):
                eng = nc.sync if st % 2 == 0 else nc.scalar
                eng.dma_start(x_stage[:, st, :], x_tiled[:, st, :])
            for kt in range(nd):
                nc.vector.tensor_copy(w_sb[:, kt, :], w_stage[:, kt, :])

            def consume_tile(st):
                # round to bf16 for the PV matmul (ScalarE: DVE stays free
                # for the xT copies)
                nc.scalar.copy(x_nat[:, st, :], x_stage[:, st, :])
                # xT[p, dt, st*128+q] = x[st*128+q, dt*128+p]
                ps = trp.tile([P, nd, P], F32, tag="tr", name="trps")
                for dt in range(nd):
                    nc.tensor.transpose(
                        ps[:, dt, :],
                        x_stage[:, st, dt * P:(dt + 1) * P],
                        ident[:],
                    )
                nc.vector.tensor_copy(xT[:, :, st * P:(st + 1) * P], ps[:])

            def xw_chunk(sb):
                # xwT[e, s-block] = sum_d W[d, e] x[s, d]
                for et in range(nd):
                    ps = mmp.tile([P, SB], F32, tag="mm", name="mmps")
                    for kt in range(nd):
                        nc.tensor.matmul(
                            ps[:],
                            w_sb[:, kt, et * P:(et + 1) * P],
                            xT[:, kt, sb * SB:(sb + 1) * SB],
                            start=(kt == 0),
                            stop=(kt == nd - 1),
                        )
                    nc.vector.tensor_copy(xwT[:, et, sb * SB:(sb + 1) * SB], ps[:])

            for st in range(nst):
                consume_tile(st)
                if st % 4 == 3:
                    xw_chunk(st // 4)

            strips = [None] * nsb
            pts = [None] * nsb
            bcs = [None] * nsb
            rrecs = [None] * nsb
            rsaccs = [None] * nsb

            def sub_exp(sb, tt):
                """shifted-exp of one staged tile (DVE sub + ScalarE exp),
                plus the DVE partial-rowsum accumulate over the exp'd tile."""
                strip = strips[sb]
                nc.vector.tensor_sub(
                    strip[:, tt, :], strip[:, tt, :], bcs[sb][:]
                )
                nc.scalar.activation(
                    pts[sb][:, tt, :],
                    strip[:, tt, :],
                    AF.Exp,
                    scale=SCALE,
                )
                # rowsum lane-partials accumulate on GpSimd (it idles;
                # DVE was gating PSUM-slot release for the PE).
                rs_new = rsaccp.tile([P, SB], F32, tag="rsacc", name="rsacc")
                if rsaccs[sb] is None:
                    nc.gpsimd.tensor_copy(rs_new[:], pts[sb][:, tt, :])
                else:
                    nc.gpsimd.tensor_add(
                        rs_new[:], pts[sb][:, tt, :], rsaccs[sb][:]
                    )
                rsaccs[sb] = rs_new

            def finish_rowsum(sb):
                """rowsum lane-partials -> PE transpose -> DVE free-axis sum
                (lands directly in [s-partition, 1] layout) -> reciprocal.
                No GpSimd involved, so it never contends with the max."""
                rtp = trp.tile([P, nss, P], F32, tag="tr", name="rtp")
                for ss in range(nss):
                    nc.tensor.transpose(
                        rtp[:, ss, :],
                        rsaccs[sb][:, ss * P:(ss + 1) * P],
                        ident[:],
                    )
                rsc = rrsbp.tile([P, nss], F32, tag="rsc", name="rsc")
                for ss in range(nss):
                    nc.vector.tensor_reduce(
                        rsc[:, ss:ss + 1], rtp[:, ss, :],
                        axis=mybir.AxisListType.X, op=mybir.AluOpType.add,
                    )
                rr = rrsbp.tile([P, nss], F32, tag="rrsb", name="rrsb")
                nc.vector.reciprocal(rr[:], rsc[:])
                rrecs[sb] = rr

            def stage_b_units(sb):
                """The PV block as a list of small emission units, so it can
                be zipped into another block's prod stream at fine grain."""
                ptt = pts[sb]
                st = {}
                units = []

                def start_chunk(ss):
                    def f():
                        st[ss] = mmp.tile([P, d], F32, tag="mm", name="mmps")
                    return f

                def mm_unit(ss, tt):
                    def f():
                        # out[s, d] = sum_t P[s, t] x[t, d]; lhsT = PT.
                        nc.tensor.matmul(
                            st[ss][:],
                            ptt[:, tt, ss * P:(ss + 1) * P],
                            x_nat[:, tt, :],
                            start=(tt == 0),
                            stop=(tt == nst - 1),
                        )
                    return f

                def norm_unit(ss):
                    def f():
                        ot = outp.tile([P, d], F32, tag="ot", name="ot")
                        # normalize on ScalarE straight from PSUM: DVE was
                        # the release path for PE PSUM slots and fell behind
                        nc.scalar.mul(ot[:], st[ss][:], rrecs[sb][:, ss:ss + 1])
                        # outputs go on the Sync queue: it is idle in steady
                        # state, while the Act engine carries copies + exps.
                        nc.sync.dma_start(o_tiled[:, sb * nss + ss, :], ot[:])
                    return f

                for ss in range(nss):
                    units.append(start_chunk(ss))
                    for tt in range(nst):
                        units.append(mm_unit(ss, tt))
                    units.append(norm_unit(ss))
                return units

            def run_prod(sb, prev, pvsb=None):
                """prodT tiles of block sb, zipped (per tile) with block
                `prev`'s sub/exp and block `pvsb`'s PV units."""
                strips[sb] = stripp.tile(
                    [P, nst, SB], F32, tag="strip", name="strip"
                )
                pts[sb] = ptp.tile([P, nst, SB], BF16, tag="pt", name="pt")
                units = stage_b_units(pvsb) if pvsb is not None else []
                ui = 0
                acc = None
                for tt in range(nst):
                    ps = mmp.tile([P, SB], F32, tag="mm")
                    for et in range(nd):
                        nc.tensor.matmul(
                            ps[:],
                            xT[:, et, tt * P:(tt + 1) * P],
                            xwT[:, et, sb * SB:(sb + 1) * SB],
                            start=(et == 0),
                            stop=(et == nd - 1),
                        )
                    # stage raw scores (ScalarE) + max-accumulate (DVE).
                    # The max chain is bf16: the shift cancels exactly in the
                    # softmax ratio, only the overflow margin moves ~0.4%.
                    nc.scalar.copy(strips[sb][:, tt, :], ps[:])
                    acc_new = accp.tile([P, SB], BF16, tag="acc")
                    if acc is None:
                        nc.vector.tensor_copy(acc_new[:], ps[:])
                    else:
                        nc.vector.tensor_max(acc_new[:], ps[:], acc[:])
                    acc = acc_new
                    if prev is not None:
                        sub_exp(prev, tt)
                    # the PV block's rowsum chain: emitted one tile into this
                    # stream so its data (complete since the previous block)
                    # is ready and the PE transposes never stall.
                    if tt == 1 and pvsb is not None:
                        finish_rowsum(pvsb)
                    # PV pops start one tile late: the first PV matmul gates
                    # on the whole PT tile (all 16 exps of that block), whose
                    # last exp only clears ScalarE shortly after this stream
                    # begins.
                    target = tt * len(units) // (nst - 1)
                    while ui < target:
                        units[ui]()
                        ui += 1
                # row max, replicated across all partitions, on idle GpSimd
                bc = bcp.tile([P, SB], BF16, tag="bc", name="bc", bufs=1)
                nc.gpsimd.partition_all_reduce(
                    bc[:], acc[:], channels=P, reduce_op=bass_isa.ReduceOp.max
                )
                bcs[sb] = bc

            # software pipeline:
            #   prod(0) | prod(1)+exp(0) | prod(2)+exp(1)+PV(0) |
            #   prod(3)+exp(2)+PV(1) | PV(2)+exp(3) | PV(3)
            run_prod(0, None)
            run_prod(1, 0)
            for sb in range(2, nsb):
                run_prod(sb, sb - 1, pvsb=sb - 2)
            finish_rowsum(nsb - 2)
            units2 = stage_b_units(nsb - 2)
            ei = 0
            for i, u in enumerate(units2):
                u()
                # front-loaded: the last block's exps must clear ScalarE
                # before its PV starts right after this stream.
                target = min(nst, (i + 1) * nst * 5 // (3 * len(units2)))
                while ei < target:
                    sub_exp(nsb - 1, ei)
                    ei += 1
            while ei < nst:
                sub_exp(nsb - 1, ei)
                ei += 1
            finish_rowsum(nsb - 1)
            for u in stage_b_units(nsb - 1):
                u()

    nc.compile()
    return nc


_NC_CACHE = {}


def _get_nc():
    if "nc" not in _NC_CACHE:
        _NC_CACHE["nc"] = build_nc()
    return _NC_CACHE["nc"]


def kernel(x: np.ndarray, attn_matrix: np.ndarray) -> np.ndarray:
    assert x.shape == (B, S, D) and attn_matrix.shape == (D, D)
    nc = _get_nc()
    w = np.ascontiguousarray(attn_matrix, dtype=np.float32)
    in_maps = [
        {"x": np.ascontiguousarray(x[b], dtype=np.float32), "w": w}
        for b in range(B)
    ]
    res = bass_utils.run_bass_kernel_spmd(nc, in_maps, core_ids=list(range(B)))
    out = np.stack([res.results[b]["o"] for b in range(B)], axis=0)
    return out.astype(np.float32, copy=False)


# revision 26
# speedup vs baseline: 1.2045x; 1.2045x over previous
"""Bass/Tile kernel for BilinearAttentionLayer on 8 NeuronCores.

out[b] = softmax(x[b] @ W @ x[b]^T / sqrt(D)) @ x[b]

Sharding: data-parallel over batch (8 batches -> 8 cores). Each core runs an
identical program on its own batch slice (x[b], W) -> out[b].

Per-core layout: the score matrix is kept transposed (scoresT[t, s]) so
every matmul operand is naturally oriented and no score-matrix transposes
are needed:
  xwT   = matmul(lhsT=W,   rhs=xT)      [e, s]
  prodT = matmul(lhsT=xT,  rhs=xwT)     [t, s]   (softmax axis = partitions)
  PT    = exp((prodT - rowmax)/sqrt(D))           (exact, safe softmax)
  out   = matmul(lhsT=PT,  rhs=x_nat)   [s, d]   (lands natural)
  rowsum rides the PV loop as N=1 matmuls against a ones column, landing
  directly in [s-partition, 1] layout for the per-partition normalization.
The only data transposes are 64 PE transposes of x itself.

Row max (softmax axis = partitions here): DVE max-accumulate across the 16
prodT PSUM tiles -> one GpSimd partition_all_reduce(max), whose output is
already replicated across all partitions -> DVE subtract on the staged raw
scores -> ScalarE Exp.  The per-row max makes the kernel robust to any
input values (the shifted exp never overflows and rowsum >= 1).

Dtypes: score path (xT, W, xwT) in float32r (1 cycle/row on the PE at
N=512); raw scores staged in f32; PT and x_nat in bf16 for the PV matmul.

Scheduling (what this revision changes vs. the first working version):
 * Input DMAs are issued up front and split across BOTH hardware DGE
   queues (SP + Activation) -- descriptor generation was the input-load
   bottleneck (2KB per descriptor, ~10 ns per descriptor per DGE).
   W goes first on the Act queue, x tiles alternate SP/Act.
 * Transposes and xw chunks are interleaved so the PE stream chases the
   DMA stream tile-by-tile and the HAM clock never re-throttles.
 * Each PV block (stage_b) is zipped into the NEXT prod block's matmul
   stream at per-tile granularity, so the ScalarE/DVE softmax work of one
   block spreads across the whole PE stream instead of bunching up and
   stalling the PV matmuls at block boundaries.
 * The last block's sub/exp is spread evenly across the previous block's
   PV stream (the old version bunched it into the first quarter and the
   final PV stalled ~5us on the exp backlog).
 * Output normalize runs on ScalarE (activation Copy with per-partition
   reciprocal scale) straight out of PSUM; output DMAs alternate queues.
"""

import numpy as np

import concourse.mybir as mybir
import concourse.tile as tile
from concourse import bacc
from concourse import bass_isa
from concourse import bass_utils
from concourse.masks import make_identity

B = 8
S = 2048
D = 512
P = 128
SB = 512  # s-block width (one fp32 PSUM bank)

F32 = mybir.dt.float32
F32R = mybir.dt.float32r
BF16 = mybir.dt.bfloat16

SCALE = float(1.0 / np.sqrt(np.float64(D)))
AF = mybir.ActivationFunctionType


def build_nc(s=S, d=D):
    nd = d // P   # d/e tiles of 128
    nst = s // P  # s/t tiles of 128
    nsb = s // SB  # s-blocks
    nss = SB // P  # 128-chunks per s-block

    nc = bacc.Bacc(
        "TRN2",
        target_bir_lowering=False,
        debug=False,
        num_devices=B,
    )
    x_d = nc.dram_tensor("x", [s, d], F32, kind="ExternalInput").ap()
    w_d = nc.dram_tensor("w", [d, d], F32, kind="ExternalInput").ap()
    o_d = nc.dram_tensor("o", [s, d], F32, kind="ExternalOutput").ap()

    x_tiled = x_d.rearrange("(n p) d -> p n d", p=P)  # [128, nst, d]
    w_tiled = w_d.rearrange("(k p) e -> p k e", p=P)  # [128, nd, d]
    o_tiled = o_d.rearrange("(n p) d -> p n d", p=P)

    with tile.TileContext(nc) as tc:
        with (
            tc.tile_pool(name="const", bufs=1) as constp,
            tc.tile_pool(name="big", bufs=1) as bigp,
            tc.tile_pool(name="strip", bufs=2) as stripp,
            tc.tile_pool(name="ptp", bufs=2) as ptp,
            tc.tile_pool(name="bcast", bufs=2) as bcp,
            tc.tile_pool(name="outs", bufs=3) as outp,
            tc.tile_pool(name="acc", bufs=2) as accp,
            tc.tile_pool(name="rsacc", bufs=2) as rsaccp,
            tc.tile_pool(name="rrsb", bufs=2) as rrsbp,
            tc.tile_pool(name="mm", bufs=6, space="PSUM") as mmp,
            tc.tile_pool(name="tr", bufs=2, space="PSUM") as trp,
        ):
            ident = constp.tile([P, P], F32)
            make_identity(nc, ident[:])
            x_nat = bigp.tile([P, nst, d], BF16)
            xT = bigp.tile([P, nd, s], F32R)
            w_sb = bigp.tile([P, nd, d], F32R)
            xwT = bigp.tile([P, nd, s], F32R)

            # ---- staging: x and W land in the strip pool's two buffers
            # (same shape, dead before the strips are first written), so
            # every input DMA can be issued up front with no buffer
            # interlock, split across both HWDGE queues.
            x_stage = stripp.tile([P, nst, SB], F32, tag="strip", name="xstg")
            w_stage = stripp.tile([P, nst, SB], F32, tag="strip", name="wstg")
            # x0/x1 first so the transposes can start ASAP; W right after
            # (needed by xw_chunk(0)); the rest of x streams behind.
            nc.sync.dma_start(x_stage[:, 0, :], x_tiled[:, 0, :])
            nc.scalar.dma_start(x_stage[:, 1, :], x_tiled[:, 1, :])
            nc.sync.dma_start(w_stage[:, 0:2, :], w_tiled[:, 0:2, :])
            nc.scalar.dma_start(w_stage[:, 2:4, :], w_tiled[:, 2:4, :])
            for st in range(2, nst):
                eng = nc.sync if st % 2 == 0 else nc.scalar
                eng.dma_start(x_stage[:, st, :], x_tiled[:, st, :])
            for kt in range(nd):
                nc.vector.tensor_copy(w_sb[:, kt, :], w_stage[:, kt, :])

            def consume_tile(st):
                # round to bf16 for the PV matmul (ScalarE: DVE stays free
                # for the xT copies)
                nc.scalar.copy(x_nat[:, st, :], x_stage[:, st, :])
                # xT[p, dt, st*128+q] = x[st*128+q, dt*128+p]
                ps = trp.tile([P, nd, P], F32, tag="tr", name="trps")
                for dt in range(nd):
                    nc.tensor.transpose(
                        ps[:, dt, :],
                        x_stage[:, st, dt * P:(dt + 1) * P],
                        ident[:],
                    )
                nc.vector.tensor_copy(xT[:, :, st * P:(st + 1) * P], ps[:])

            def xw_chunk(sb):
                # xwT[e, s-block] = sum_d W[d, e] x[s, d]
                for et in range(nd):
                    ps = mmp.tile([P, SB], F32, tag="mm", name="mmps")
                    for kt in range(nd):
                        nc.tensor.matmul(
                            ps[:],
                            w_sb[:, kt, et * P:(et + 1) * P],
                            xT[:, kt, sb * SB:(sb + 1) * SB],
                            start=(kt == 0),
                            stop=(kt == nd - 1),
                        )
                    nc.vector.tensor_copy(xwT[:, et, sb * SB:(sb + 1) * SB], ps[:])

            for st in range(nst):
                consume_tile(st)
                if st % 4 == 3:
                    xw_chunk(st // 4)

            strips = [None] * nsb
            pts = [None] * nsb
            bcs = [None] * nsb
            rrecs = [None] * nsb
            rsaccs = [None] * nsb

            def sub_exp(sb, tt):
                """shifted-exp of one staged tile (DVE sub + ScalarE exp),
                plus the DVE partial-rowsum accumulate over the exp'd tile."""
                strip = strips[sb]
                nc.vector.tensor_sub(
                    strip[:, tt, :], strip[:, tt, :], bcs[sb][:]
                )
                nc.scalar.activation(
                    pts[sb][:, tt, :],
                    strip[:, tt, :],
                    AF.Exp,
                    scale=SCALE,
                )
                # rowsum lane-partials accumulate on DVE (GpSimd tensor ops
                # measure ~1.4us per [128,512] tile -- 5x slower than DVE)
                rs_new = rsaccp.tile([P, SB], F32, tag="rsacc", name="rsacc")
                if rsaccs[sb] is None:
                    nc.vector.tensor_copy(rs_new[:], pts[sb][:, tt, :])
                else:
                    nc.vector.tensor_add(
                        rs_new[:], pts[sb][:, tt, :], rsaccs[sb][:]
                    )
                rsaccs[sb] = rs_new

            def finish_rowsum(sb):
                """rowsum lane-partials -> PE transpose -> DVE free-axis sum
                (lands directly in [s-partition, 1] layout) -> reciprocal.
                No GpSimd involved, so it never contends with the max."""
                rtp = trp.tile([P, nss, P], F32, tag="tr", name="rtp")
                for ss in range(nss):
                    nc.tensor.transpose(
                        rtp[:, ss, :],
                        rsaccs[sb][:, ss * P:(ss + 1) * P],
                        ident[:],
                    )
                rsc = rrsbp.tile([P, nss], F32, tag="rsc", name="rsc")
                for ss in range(nss):
                    nc.vector.tensor_reduce(
                        rsc[:, ss:ss + 1], rtp[:, ss, :],
                        axis=mybir.AxisListType.X, op=mybir.AluOpType.add,
                    )
                rr = rrsbp.tile([P, nss], F32, tag="rrsb", name="rrsb")
                nc.vector.reciprocal(rr[:], rsc[:])
                rrecs[sb] = rr

            def stage_b_units(sb):
                """The PV block as a list of small emission units, so it can
                be zipped into another block's prod stream at fine grain."""
                ptt = pts[sb]
                st = {}
                units = []

                def start_chunk(ss):
                    def f():
                        st[ss] = mmp.tile([P, d], F32, tag="mm", name="mmps")
                    return f

                def mm_unit(ss, tt):
                    def f():
                        # out[s, d] = sum_t P[s, t] x[t, d]; lhsT = PT.
                        nc.tensor.matmul(
                            st[ss][:],
                            ptt[:, tt, ss * P:(ss + 1) * P],
                            x_nat[:, tt, :],
                            start=(tt == 0),
                            stop=(tt == nst - 1),
                        )
                    return f

                def norm_unit(ss):
                    def f():
                        ot = outp.tile([P, d], F32, tag="ot", name="ot")
                        # normalize on ScalarE straight from PSUM: DVE was
                        # the release path for PE PSUM slots and fell behind
                        nc.scalar.mul(ot[:], st[ss][:], rrecs[sb][:, ss:ss + 1])
                        # outputs go on the Sync queue: it is idle in steady
                        # state, while the Act engine carries copies + exps.
                        nc.sync.dma_start(o_tiled[:, sb * nss + ss, :], ot[:])
                    return f

                for ss in range(nss):
                    units.append(start_chunk(ss))
                    for tt in range(nst):
                        units.append(mm_unit(ss, tt))
                    units.append(norm_unit(ss))
                return units

            def run_prod(sb, prev, pvsb=None):
                """prodT tiles of block sb, zipped (per tile) with block
                `prev`'s sub/exp and block `pvsb`'s PV units."""
                strips[sb] = stripp.tile(
                    [P, nst, SB], F32, tag="strip", name="strip"
                )
                pts[sb] = ptp.tile([P, nst, SB], BF16, tag="pt", name="pt")
                units = stage_b_units(pvsb) if pvsb is not None else []
                ui = 0
                acc = None
                for tt in range(nst):
                    ps = mmp.tile([P, SB], F32, tag="mm")
                    for et in range(nd):
                        nc.tensor.matmul(
                            ps[:],
                            xT[:, et, tt * P:(tt + 1) * P],
                            xwT[:, et, sb * SB:(sb + 1) * SB],
                            start=(et == 0),
                            stop=(et == nd - 1),
                        )
                    # stage raw scores (ScalarE) + max-accumulate (DVE).
                    # The max chain is bf16: the shift cancels exactly in the
                    # softmax ratio, only the overflow margin moves ~0.4%.
                    nc.scalar.copy(strips[sb][:, tt, :], ps[:])
                    acc_new = accp.tile([P, SB], BF16, tag="acc")
                    if acc is None:
                        nc.vector.tensor_copy(acc_new[:], ps[:])
                    else:
                        nc.vector.tensor_max(acc_new[:], ps[:], acc[:])
                    acc = acc_new
                    if prev is not None:
                        sub_exp(prev, tt)
                    # the PV block's rowsum chain: emitted one tile into this
                    # stream so its data (complete since the previous block)
                    # is ready and the PE transposes never stall.
                    if tt == 1 and pvsb is not None:
                        finish_rowsum(pvsb)
                    # PV pops start one tile late: the first PV matmul gates
                    # on the whole PT tile (all 16 exps of that block), whose
                    # last exp only clears ScalarE shortly after this stream
                    # begins.
                    target = tt * len(units) // (nst - 1)
                    while ui < target:
                        units[ui]()
                        ui += 1
                # row max, replicated across all partitions, on idle GpSimd
                bc = bcp.tile([P, SB], BF16, tag="bc", name="bc", bufs=1)
                nc.gpsimd.partition_all_reduce(
                    bc[:], acc[:], channels=P, reduce_op=bass_isa.ReduceOp.max
                )
                bcs[sb] = bc

            # software pipeline:
            #   prod(0) | prod(1)+exp(0) | prod(2)+exp(1)+PV(0) |
            #   prod(3)+exp(2)+PV(1) | PV(2)+exp(3) | PV(3)
            run_prod(0, None)
            run_prod(1, 0)
            for sb in range(2, nsb):
                run_prod(sb, sb - 1, pvsb=sb - 2)
            finish_rowsum(nsb - 2)
            units2 = stage_b_units(nsb - 2)
            ei = 0
            for i, u in enumerate(units2):
                u()
                # front-loaded: the last block's exps must clear ScalarE
                # before its PV starts right after this stream.
                target = min(nst, (i + 1) * nst * 5 // (3 * len(units2)))
                while ei < target:
                    sub_exp(nsb - 1, ei)
                    ei += 1
            while ei < nst:
                sub_exp(nsb - 1, ei)
                ei += 1
            finish_rowsum(nsb - 1)
            for u in stage_b_units(nsb - 1):
                u()

    nc.compile()
    return nc


_NC_CACHE = {}


def _get_nc():
    if "nc" not in _NC_CACHE:
        _NC_CACHE["nc"] = build_nc()
    return _NC_CACHE["nc"]


def kernel(x: np.ndarray, attn_matrix: np.ndarray) -> np.ndarray:
    assert x.shape == (B, S, D) and attn_matrix.shape == (D, D)
    nc = _get_nc()
    w = np.ascontiguousarray(attn_matrix, dtype=np.float32)
    in_maps = [
        {"x": np.ascontiguousarray(x[b], dtype=np.float32), "w": w}
        for b in range(B)
    ]
    res = bass_utils.run_bass_kernel_spmd(nc, in_maps, core_ids=list(range(B)))
    out = np.stack([res.results[b]["o"] for b in range(B)], axis=0)
    return out.astype(np.float32, copy=False)
